# revision 14
# baseline (speedup 1.0000x reference)
"""Expert-parallel MoE FFN kernel for Trainium2 (8 NeuronCores).

Strategy: each of the 8 experts lives on its own core. Rows are routed
host-side (argsort by note_type_pos), padded to a uniform capacity C,
and shipped feature-major (transposed) so the device kernel is a pure
dense 2-layer MLP with the feature dimension on SBUF partitions:

    hT = relu(W1.T @ xT + b1)     [F, C]
    yT = W2.T @ hT + b2           [H, C]

x and the weights are shipped in bf16 (error ~3e-3 vs the 2e-2 gate;
half the HBM traffic of f32); biases, PSUM accumulation and the output
stay f32. Each logical block moves as ONE 3D-strided DMA descriptor —
descriptor issue costs ~600ns serialized on the DGE path regardless of
size, so many small descriptors throttle the startup ramp. Weights are
streamed through SBUF once (F blocked into 8 blocks of 512); xT and the
y accumulator stay resident. No collectives needed.
"""

import sys

sys.path.insert(0, "/opt/trn_rl_repo")

import numpy as np

import concourse.bass as bass
import concourse.mybir as mybir
from concourse import bacc
from concourse.tile import TileContext

H = 1024
F = 4096
N_EXPERTS = 8
P = 128
KH = H // P   # 8
KF = F // P   # 32
FB = 512      # F block size (weights streamed per block)
NFB = F // FB  # 8
FC = FB // P  # 4


def _row_tiles(C):
    """Split C columns into equal chunks <=512 (multiples of 16)."""
    n = -(-C // 512)
    rw = -(-C // n)
    rw = ((rw + 15) // 16) * 16
    tiles = []
    s = 0
    while s < C:
        w = min(rw, C - s)
        tiles.append((s, w))
        s += w
    return tiles


def build_expert_kernel(C, reps=1, n_wu=6):
    """One expert's 2-layer MLP: xT [H, C] -> yT [H, C].

    w1 arrives host-packed as [P, NFB*FC*KH*128] (partition-major blocked:
    per partition p, element (fb, fc, k, j) = W1[k*128+p, fb*512+fc*128+j])
    so every load slice is contiguous per partition — no sub-512B runs.
    """
    f32 = mybir.dt.float32
    bf16 = mybir.dt.bfloat16
    nc = bacc.Bacc(None, target_bir_lowering=False)
    xT = nc.dram_tensor("xT", [H, C], bf16, kind="ExternalInput")
    w1 = nc.dram_tensor("w1", [P, NFB * FC * KH * P], bf16,
                        kind="ExternalInput")
    b1v = nc.dram_tensor("b1v", [P, KF], f32, kind="ExternalInput")
    w2 = nc.dram_tensor("w2", [F, H], bf16, kind="ExternalInput")
    b2v = nc.dram_tensor("b2v", [P, KH], f32, kind="ExternalInput")
    yT = nc.dram_tensor("yT", [H, C], bf16, kind="ExternalOutput")

    # partition-major views: one DMA descriptor per logical block
    w2r = w2.rearrange("(f p) h -> p f h", p=P)   # [P, KF, H]
    xTr = xT.rearrange("(k p) c -> p k c", p=P)   # [P, KH, C]
    yTr = yT.rearrange("(m p) c -> p m c", p=P)   # [P, KH, C]

    FBW = FC * KH * P  # 4096 packed elements per fb block

    tiles = _row_tiles(C)
    rwmax = max(t[1] for t in tiles)

    with TileContext(nc) as tc:
        with (
            tc.tile_pool(name="consts", bufs=1) as consts,
            tc.tile_pool(name="xp", bufs=1) as xp,
            tc.tile_pool(name="yaccp", bufs=1) as yaccp,
            tc.tile_pool(name="youtp", bufs=1) as youtp,
            tc.tile_pool(name="w1p", bufs=3) as w1p,
            tc.tile_pool(name="w2p", bufs=3) as w2p,
            tc.tile_pool(name="hp", bufs=3) as hp,
            tc.tile_pool(name="psh", bufs=4, space="PSUM") as psh,
            tc.tile_pool(name="psy", bufs=4, space="PSUM") as psy,
        ):
            # Startup critical path, in consumption order: the first psum
            # group needs w1[fb0, fc0] + xT tile0 only; later chunks land
            # just ahead of the PE groups that consume them.
            r0_, rw_ = tiles[0]
            KHP = KH * P
            w1blk0 = w1p.tile([P, FBW], bf16, tag="w1blk")
            nc.sync.dma_start(w1blk0[:, 0:KHP], w1[:, 0:KHP])
            xT_sb = xp.tile([P, KH, C], bf16, tag="xT")
            # tile0 lands in two k-halves so the first psum groups can
            # start on k0-3 while k4-7 is still in flight
            nc.sync.dma_start(xT_sb[:, 0:KH // 2, r0_:r0_ + rw_],
                              xTr[:, 0:KH // 2, r0_:r0_ + rw_])
            nc.sync.dma_start(xT_sb[:, KH // 2:KH, r0_:r0_ + rw_],
                              xTr[:, KH // 2:KH, r0_:r0_ + rw_])
            nc.sync.dma_start(w1blk0[:, KHP:2 * KHP], w1[:, KHP:2 * KHP])
            b1_sb = consts.tile([P, KF], f32, tag="b1")
            nc.sync.dma_start(b1_sb[:], b1v[:, :])
            b2_sb = consts.tile([P, KH], f32, tag="b2")
            nc.sync.dma_start(b2_sb[:], b2v[:, :])
            nc.sync.dma_start(w1blk0[:, 2 * KHP:4 * KHP],
                              w1[:, 2 * KHP:4 * KHP])
            w2blk0 = w2p.tile([P, FC, H], bf16, tag="w2blk")
            nc.sync.dma_start(w2blk0[:, :, 0:H // 2],
                              w2r[:, 0:FC, 0:H // 2])
            nc.sync.dma_start(w2blk0[:, :, H // 2:H],
                              w2r[:, 0:FC, H // 2:H])
            for (r0, rw) in tiles[1:]:
                nc.sync.dma_start(xT_sb[:, :, r0:r0 + rw],
                                  xTr[:, :, r0:r0 + rw])

            def load_w1(fb):
                w1blk = w1p.tile([P, FBW], bf16, tag="w1blk")
                nc.sync.dma_start(w1blk[:, :],
                                  w1[:, fb * FBW:(fb + 1) * FBW])
                return w1blk

            def load_w2(fb):
                w2blk = w2p.tile([P, FC, H], bf16, tag="w2blk")
                nc.sync.dma_start(w2blk[:, :, :],
                                  w2r[:, fb * FC:(fb + 1) * FC, :])
                return w2blk

            # PE warmup during the startup DMA window: absorbs the
            # p-state ramp so real matmuls start at full clock
            if n_wu:
                wu = consts.tile([P, 512], bf16, tag="wu")
                nc.vector.memset(wu[:], 0.0)
                wups = psh.tile([P, 512], f32, tag="ph")
                for i in range(n_wu):
                    nc.tensor.matmul(wups[:], wu[:, 0:P], wu[:],
                                     start=(i == 0), stop=(i == n_wu - 1))

            yacc = yaccp.tile([P, KH, C], f32, tag="yacc")
            yout = youtp.tile([P, KH, C], bf16, tag="yout")

            def layer1(fb, w1blk, r0, rw, khalf_major=False):
                h_sb = hp.tile([P, FC, rwmax], bf16, tag="h")
                if khalf_major:
                    # startup path: sweep k0-3 over all fc groups first so
                    # the PE isn't gated on the second xT half
                    phs = [psh.tile([P, rw], f32, tag="ph")
                           for _ in range(FC)]
                    for kh in range(2):
                        for fc in range(FC):
                            for k in range(kh * KH // 2,
                                           (kh + 1) * KH // 2):
                                o = fc * KHP + k * P
                                nc.tensor.matmul(
                                    phs[fc][:],
                                    w1blk[:, o:o + P],
                                    xT_sb[:, k, r0:r0 + rw],
                                    start=(k == 0), stop=(k == KH - 1))
                    for fc in range(FC):
                        nc.scalar.activation(
                            h_sb[:, fc, :rw], phs[fc][:],
                            mybir.ActivationFunctionType.Relu,
                            bias=b1_sb[:, fb * FC + fc:fb * FC + fc + 1])
                    return h_sb
                for fc in range(FC):
                    ph = psh.tile([P, rw], f32, tag="ph")
                    for k in range(KH):
                        o = fc * KHP + k * P
                        nc.tensor.matmul(
                            ph[:],
                            w1blk[:, o:o + P],
                            xT_sb[:, k, r0:r0 + rw],
                            start=(k == 0), stop=(k == KH - 1))
                    nc.scalar.activation(
                        h_sb[:, fc, :rw], ph[:],
                        mybir.ActivationFunctionType.Relu,
                        bias=b1_sb[:, fb * FC + fc:fb * FC + fc + 1])
                return h_sb

            def layer2(fb, w2blk, h_sb, m, r0, rw, last, final_tile):
                py = psy.tile([P, rw], f32, tag="py")
                for fc in range(FC):
                    nc.tensor.matmul(
                        py[:],
                        w2blk[:, fc, m * P:(m + 1) * P],
                        h_sb[:, fc, :rw],
                        start=(fc == 0), stop=(fc == FC - 1))
                if fb == 0:
                    # fold the layer-2 bias into the first partial
                    nc.scalar.activation(
                        yacc[:, m, r0:r0 + rw], py[:],
                        mybir.ActivationFunctionType.Identity,
                        bias=b2_sb[:, m:m + 1])
                elif fb < NFB - 1:
                    nc.vector.tensor_add(
                        out=yacc[:, m, r0:r0 + rw],
                        in0=yacc[:, m, r0:r0 + rw], in1=py[:])
                else:
                    # final accumulation narrows to the bf16 output stage
                    nc.vector.tensor_add(
                        out=yout[:, m, r0:r0 + rw],
                        in0=yacc[:, m, r0:r0 + rw], in1=py[:])
                    if not last:
                        return
                    # stream writebacks behind the adds: pairs on the final
                    # tile (short drain), halves elsewhere
                    if final_tile and m % 2 == 1:
                        nc.sync.dma_start(
                            yTr[:, m - 1:m + 1, r0:r0 + rw],
                            yout[:, m - 1:m + 1, r0:r0 + rw])
                    elif not final_tile and m in (KH // 2 - 1, KH - 1):
                        m0 = 0 if m == KH // 2 - 1 else KH // 2
                        nc.sync.dma_start(
                            yTr[:, m0:m + 1, r0:r0 + rw],
                            yout[:, m0:m + 1, r0:r0 + rw])

            def body(first_blks=None, last=True):
                for fb in range(NFB):
                    if fb == 0 and first_blks is not None:
                        w1blk, w2blk = first_blks
                    else:
                        w1blk = load_w1(fb)
                        w2blk = load_w2(fb)
                    for ti, (r0, rw) in enumerate(tiles):
                        h_sb = layer1(fb, w1blk, r0, rw,
                                      khalf_major=(fb == 0 and ti == 0
                                                   and first_blks
                                                   is not None))
                        for m in range(KH):
                            layer2(fb, w2blk, h_sb, m, r0, rw, last,
                                   ti == len(tiles) - 1)

            first_blks = (w1blk0, w2blk0)
            for i in range(reps - 1):
                body(first_blks if i == 0 else None, last=False)
            body(first_blks if reps == 1 else None, last=True)
    nc.finalize()
    return nc


# SBUF residency (xT bf16 + yacc f32 at 48*C B/partition) caps capacity.
MAX_C = 1536


def _prepare(x, note_type_pos, W1, b1, W2, b2, cap):
    """Host-side routing: sort rows by expert, pad to capacity C (<= cap)."""
    import ml_dtypes
    bf16 = ml_dtypes.bfloat16
    ntp = np.asarray(note_type_pos).astype(np.int64)
    x = np.ascontiguousarray(np.asarray(x, dtype=np.float32))
    counts = np.bincount(ntp, minlength=N_EXPERTS)
    C = min(int(counts.max()), cap)
    C = max(16, ((C + 15) // 16) * 16)  # 16-aligned, no extra row-tile padding

    order = np.argsort(ntp, kind="stable")
    weights = []
    for e in range(N_EXPERTS):
        # pack W1 partition-major blocked: [P, (fb, fc, k, j)] so device
        # loads are contiguous per partition at any chunking granularity
        w1p_ = (np.asarray(W1[e]).astype(bf16)
                .reshape(KH, P, NFB, FC, P)
                .transpose(1, 2, 3, 0, 4)
                .reshape(P, NFB * FC * KH * P))
        weights.append({
            "w1": np.ascontiguousarray(w1p_),
            "b1v": np.ascontiguousarray(
                np.asarray(b1[e], dtype=np.float32).reshape(KF, P).T),
            "w2": np.ascontiguousarray(np.asarray(W2[e]).astype(bf16)),
            "b2v": np.ascontiguousarray(
                np.asarray(b2[e], dtype=np.float32).reshape(KH, P).T),
        })
    # chunk each expert's rows into groups of <= C; one SPMD launch per group
    launches = []
    off = 0
    expert_rows = []
    for e in range(N_EXPERTS):
        expert_rows.append(order[off:off + counts[e]])
        off += counts[e]
    n_launch = max(1, -(-int(counts.max()) // C))
    for g in range(n_launch):
        in_maps, row_idx = [], []
        for e in range(N_EXPERTS):
            rows = expert_rows[e][g * C:(g + 1) * C]
            row_idx.append(rows)
            xe = np.zeros((C, H), dtype=np.float32)
            if len(rows):
                xe[:len(rows)] = x[rows]
            in_maps.append({"xT": np.ascontiguousarray(xe.T.astype(bf16)),
                            **weights[e]})
        launches.append((in_maps, row_idx))
    return launches, C


def kernel(x, note_type_pos, W1, b1, W2, b2):
    launches, C = _prepare(x, note_type_pos, W1, b1, W2, b2, cap=MAX_C)
    nc = build_expert_kernel(C)
    from concourse.bass_utils import run_bass_kernel_spmd
    T = np.asarray(x).shape[0]
    out = np.zeros((T, H), dtype=np.float32)
    for in_maps, row_idx in launches:
        res = run_bass_kernel_spmd(nc, in_maps, core_ids=list(range(N_EXPERTS)))
        for e in range(N_EXPERTS):
            rows = row_idx[e]
            if len(rows):
                out[rows] = res.results[e]["yT"].T[:len(rows)].astype(
                    np.float32)
    return out


# revision 17
# speedup vs baseline: 1.2935x; 1.2935x over previous
"""Expert-parallel MoE FFN kernel for Trainium2 (8 NeuronCores).

Strategy: each of the 8 experts lives on its own core. Rows are routed
host-side (argsort by note_type_pos), padded to a uniform capacity C,
and shipped feature-major (transposed) so the device kernel is a pure
dense 2-layer MLP with the feature dimension on SBUF partitions:

    hT = relu(W1.T @ xT + b1)     [F, C]
    yT = W2.T @ hT + b2           [H, C]

x and the weights are shipped in bf16 (error ~3e-3 vs the 2e-2 gate;
half the HBM traffic of f32); biases, PSUM accumulation and the output
stay f32. Each logical block moves as ONE 3D-strided DMA descriptor —
descriptor issue costs ~600ns serialized on the DGE path regardless of
size, so many small descriptors throttle the startup ramp. Weights are
streamed through SBUF once (F blocked into 8 blocks of 512); xT and the
y accumulator stay resident. No collectives needed.
"""

import sys

sys.path.insert(0, "/opt/trn_rl_repo")

import numpy as np

import concourse.bass as bass
import concourse.mybir as mybir
from concourse import bacc
from concourse.tile import TileContext

H = 1024
F = 4096
N_EXPERTS = 8
P = 128
KH = H // P   # 8
KF = F // P   # 32
FB = 512      # F block size (weights streamed per block)
NFB = F // FB  # 8
FC = FB // P  # 4


def _row_tiles(C):
    """Split C columns into equal chunks <=512 (multiples of 16)."""
    n = -(-C // 512)
    rw = -(-C // n)
    rw = ((rw + 15) // 16) * 16
    tiles = []
    s = 0
    while s < C:
        w = min(rw, C - s)
        tiles.append((s, w))
        s += w
    return tiles


def build_expert_kernel(C, reps=1, n_wu=6):
    """One expert's 2-layer MLP: xT [H, C] -> yT [H, C].

    w1 arrives host-packed as [P, NFB*FC*KH*128] (partition-major blocked:
    per partition p, element (fb, fc, k, j) = W1[k*128+p, fb*512+fc*128+j])
    so every load slice is contiguous per partition — no sub-512B runs.
    """
    f32 = mybir.dt.float32
    bf16 = mybir.dt.bfloat16
    nc = bacc.Bacc(None, target_bir_lowering=False)
    xT = nc.dram_tensor("xT", [H, C], bf16, kind="ExternalInput")
    w1 = nc.dram_tensor("w1", [P, NFB * FC * KH * P], bf16,
                        kind="ExternalInput")
    b1v = nc.dram_tensor("b1v", [P, KF], f32, kind="ExternalInput")
    w2 = nc.dram_tensor("w2", [F, H], bf16, kind="ExternalInput")
    b2v = nc.dram_tensor("b2v", [P, KH], f32, kind="ExternalInput")
    yT = nc.dram_tensor("yT", [H, C], bf16, kind="ExternalOutput")

    # partition-major views: one DMA descriptor per logical block
    w2r = w2.rearrange("(f p) h -> p f h", p=P)   # [P, KF, H]
    xTr = xT.rearrange("(k p) c -> p k c", p=P)   # [P, KH, C]
    yTr = yT.rearrange("(m p) c -> p m c", p=P)   # [P, KH, C]

    FBW = FC * KH * P  # 4096 packed elements per fb block

    tiles = _row_tiles(C)
    rwmax = max(t[1] for t in tiles)

    with TileContext(nc) as tc:
        with (
            tc.tile_pool(name="consts", bufs=1) as consts,
            tc.tile_pool(name="xp", bufs=1) as xp,
            tc.tile_pool(name="yaccp", bufs=1) as yaccp,
            tc.tile_pool(name="youtp", bufs=1) as youtp,
            tc.tile_pool(name="w1p", bufs=3) as w1p,
            tc.tile_pool(name="w2p", bufs=3) as w2p,
            tc.tile_pool(name="hp", bufs=3) as hp,
            tc.tile_pool(name="psh", bufs=4, space="PSUM") as psh,
            tc.tile_pool(name="psy", bufs=4, space="PSUM") as psy,
        ):
            # Startup critical path, in consumption order: the first psum
            # group needs w1[fb0, fc0] + xT tile0 only; later chunks land
            # just ahead of the PE groups that consume them.
            r0_, rw_ = tiles[0]
            KHP = KH * P
            w1blk0 = w1p.tile([P, FBW], bf16, tag="w1blk")
            nc.sync.dma_start(w1blk0[:, 0:KHP], w1[:, 0:KHP])
            xT_sb = xp.tile([P, KH, C], bf16, tag="xT")
            # tile0 lands in two k-halves so the first psum groups can
            # start on k0-3 while k4-7 is still in flight
            nc.sync.dma_start(xT_sb[:, 0:KH // 2, r0_:r0_ + rw_],
                              xTr[:, 0:KH // 2, r0_:r0_ + rw_])
            nc.sync.dma_start(xT_sb[:, KH // 2:KH, r0_:r0_ + rw_],
                              xTr[:, KH // 2:KH, r0_:r0_ + rw_])
            nc.sync.dma_start(w1blk0[:, KHP:2 * KHP], w1[:, KHP:2 * KHP])
            b1_sb = consts.tile([P, KF], f32, tag="b1")
            nc.sync.dma_start(b1_sb[:], b1v[:, :])
            b2_sb = consts.tile([P, KH], f32, tag="b2")
            nc.sync.dma_start(b2_sb[:], b2v[:, :])
            nc.sync.dma_start(w1blk0[:, 2 * KHP:4 * KHP],
                              w1[:, 2 * KHP:4 * KHP])
            w2blk0 = w2p.tile([P, FC, H], bf16, tag="w2blk")
            nc.sync.dma_start(w2blk0[:, :, 0:H // 2],
                              w2r[:, 0:FC, 0:H // 2])
            nc.sync.dma_start(w2blk0[:, :, H // 2:H],
                              w2r[:, 0:FC, H // 2:H])
            for (r0, rw) in tiles[1:]:
                nc.sync.dma_start(xT_sb[:, :, r0:r0 + rw],
                                  xTr[:, :, r0:r0 + rw])

            def load_w1(fb):
                w1blk = w1p.tile([P, FBW], bf16, tag="w1blk")
                nc.sync.dma_start(w1blk[:, :],
                                  w1[:, fb * FBW:(fb + 1) * FBW])
                return w1blk

            def load_w2(fb):
                w2blk = w2p.tile([P, FC, H], bf16, tag="w2blk")
                nc.sync.dma_start(w2blk[:, :, :],
                                  w2r[:, fb * FC:(fb + 1) * FC, :])
                return w2blk

            # PE warmup during the startup DMA window: absorbs the
            # p-state ramp so real matmuls start at full clock
            if n_wu:
                wu = consts.tile([P, 512], bf16, tag="wu")
                nc.vector.memset(wu[:], 0.0)
                wups = psh.tile([P, 512], f32, tag="ph")
                for i in range(n_wu):
                    nc.tensor.matmul(wups[:], wu[:, 0:P], wu[:],
                                     start=(i == 0), stop=(i == n_wu - 1))
                # touch Relu now so the act-table load (~1.3us) happens
                # inside the DMA window, not on the first real activation
                wuact = consts.tile([P, 16], f32, tag="wuact")
                nc.scalar.activation(wuact[:], wups[:, 0:16],
                                     mybir.ActivationFunctionType.Relu)

            yacc = yaccp.tile([P, KH, C], f32, tag="yacc")
            yout = youtp.tile([P, KH, C], bf16, tag="yout")

            def layer1(fb, w1blk, r0, rw, khalf_major=False):
                h_sb = hp.tile([P, FC, rwmax], bf16, tag="h")
                if khalf_major:
                    # startup path: sweep k0-3 over all fc groups first so
                    # the PE isn't gated on the second xT half
                    phs = []
                    for fci in range(FC):
                        ph_i = psh.tile([P, rw], f32, tag="ph",
                                        name=f"ph_s{fci}")
                        phs.append(ph_i)
                    for kh in range(2):
                        for fc in range(FC):
                            for k in range(kh * KH // 2,
                                           (kh + 1) * KH // 2):
                                o = fc * KHP + k * P
                                nc.tensor.matmul(
                                    phs[fc][:],
                                    w1blk[:, o:o + P],
                                    xT_sb[:, k, r0:r0 + rw],
                                    start=(k == 0), stop=(k == KH - 1))
                    for fc in range(FC):
                        nc.scalar.activation(
                            h_sb[:, fc, :rw], phs[fc][:],
                            mybir.ActivationFunctionType.Relu,
                            bias=b1_sb[:, fb * FC + fc:fb * FC + fc + 1])
                    return h_sb
                for fc in range(FC):
                    ph = psh.tile([P, rw], f32, tag="ph")
                    for k in range(KH):
                        o = fc * KHP + k * P
                        nc.tensor.matmul(
                            ph[:],
                            w1blk[:, o:o + P],
                            xT_sb[:, k, r0:r0 + rw],
                            start=(k == 0), stop=(k == KH - 1))
                    nc.scalar.activation(
                        h_sb[:, fc, :rw], ph[:],
                        mybir.ActivationFunctionType.Relu,
                        bias=b1_sb[:, fb * FC + fc:fb * FC + fc + 1])
                return h_sb

            def layer2(fb, w2blk, h_sb, m, r0, rw, last, final_tile):
                py = psy.tile([P, rw], f32, tag="py")
                for fc in range(FC):
                    nc.tensor.matmul(
                        py[:],
                        w2blk[:, fc, m * P:(m + 1) * P],
                        h_sb[:, fc, :rw],
                        start=(fc == 0), stop=(fc == FC - 1))
                if fb == 0:
                    # fold the layer-2 bias into the first partial
                    nc.scalar.activation(
                        yacc[:, m, r0:r0 + rw], py[:],
                        mybir.ActivationFunctionType.Identity,
                        bias=b2_sb[:, m:m + 1])
                elif fb < NFB - 1:
                    nc.vector.tensor_add(
                        out=yacc[:, m, r0:r0 + rw],
                        in0=yacc[:, m, r0:r0 + rw], in1=py[:])
                else:
                    # final accumulation narrows to the bf16 output stage
                    nc.vector.tensor_add(
                        out=yout[:, m, r0:r0 + rw],
                        in0=yacc[:, m, r0:r0 + rw], in1=py[:])
                    if not last:
                        return
                    # stream writebacks behind the adds; the final tile
                    # tapers (half, then 3, then 1) so the last descriptor
                    # after the last add is as small as possible
                    if final_tile and m in (3, 6, 7):
                        m0 = {3: 0, 6: 4, 7: 7}[m]
                        nc.sync.dma_start(
                            yTr[:, m0:m + 1, r0:r0 + rw],
                            yout[:, m0:m + 1, r0:r0 + rw])
                    elif not final_tile and m in (KH // 2 - 1, KH - 1):
                        m0 = 0 if m == KH // 2 - 1 else KH // 2
                        nc.sync.dma_start(
                            yTr[:, m0:m + 1, r0:r0 + rw],
                            yout[:, m0:m + 1, r0:r0 + rw])

            def body(first_blks=None, last=True):
                for fb in range(NFB):
                    if fb == 0 and first_blks is not None:
                        w1blk, w2blk = first_blks
                    else:
                        w1blk = load_w1(fb)
                        w2blk = load_w2(fb)
                    for ti, (r0, rw) in enumerate(tiles):
                        h_sb = layer1(fb, w1blk, r0, rw,
                                      khalf_major=(fb == 0 and ti == 0
                                                   and first_blks
                                                   is not None))
                        for m in range(KH):
                            layer2(fb, w2blk, h_sb, m, r0, rw, last,
                                   ti == len(tiles) - 1)

            first_blks = (w1blk0, w2blk0)
            for i in range(reps - 1):
                body(first_blks if i == 0 else None, last=False)
            body(first_blks if reps == 1 else None, last=True)
    nc.finalize()
    return nc


# SBUF residency (xT bf16 + yacc f32 at 48*C B/partition) caps capacity.
MAX_C = 1536


def _prepare(x, note_type_pos, W1, b1, W2, b2, cap):
    """Host-side routing: sort rows by expert, pad to capacity C (<= cap)."""
    import ml_dtypes
    bf16 = ml_dtypes.bfloat16
    ntp = np.asarray(note_type_pos).astype(np.int64)
    x = np.ascontiguousarray(np.asarray(x, dtype=np.float32))
    counts = np.bincount(ntp, minlength=N_EXPERTS)
    C = min(int(counts.max()), cap)
    C = max(16, ((C + 15) // 16) * 16)  # 16-aligned, no extra row-tile padding

    order = np.argsort(ntp, kind="stable")
    weights = []
    for e in range(N_EXPERTS):
        # pack W1 partition-major blocked: [P, (fb, fc, k, j)] so device
        # loads are contiguous per partition at any chunking granularity
        w1p_ = (np.asarray(W1[e]).astype(bf16)
                .reshape(KH, P, NFB, FC, P)
                .transpose(1, 2, 3, 0, 4)
                .reshape(P, NFB * FC * KH * P))
        weights.append({
            "w1": np.ascontiguousarray(w1p_),
            "b1v": np.ascontiguousarray(
                np.asarray(b1[e], dtype=np.float32).reshape(KF, P).T),
            "w2": np.ascontiguousarray(np.asarray(W2[e]).astype(bf16)),
            "b2v": np.ascontiguousarray(
                np.asarray(b2[e], dtype=np.float32).reshape(KH, P).T),
        })
    # chunk each expert's rows into groups of <= C; one SPMD launch per group
    launches = []
    off = 0
    expert_rows = []
    for e in range(N_EXPERTS):
        expert_rows.append(order[off:off + counts[e]])
        off += counts[e]
    n_launch = max(1, -(-int(counts.max()) // C))
    for g in range(n_launch):
        in_maps, row_idx = [], []
        for e in range(N_EXPERTS):
            rows = expert_rows[e][g * C:(g + 1) * C]
            row_idx.append(rows)
            xe = np.zeros((C, H), dtype=np.float32)
            if len(rows):
                xe[:len(rows)] = x[rows]
            in_maps.append({"xT": np.ascontiguousarray(xe.T.astype(bf16)),
                            **weights[e]})
        launches.append((in_maps, row_idx))
    return launches, C


def kernel(x, note_type_pos, W1, b1, W2, b2):
    launches, C = _prepare(x, note_type_pos, W1, b1, W2, b2, cap=MAX_C)
    nc = build_expert_kernel(C)
    from concourse.bass_utils import run_bass_kernel_spmd
    T = np.asarray(x).shape[0]
    out = np.zeros((T, H), dtype=np.float32)
    for in_maps, row_idx in launches:
        res = run_bass_kernel_spmd(nc, in_maps, core_ids=list(range(N_EXPERTS)))
        for e in range(N_EXPERTS):
            rows = row_idx[e]
            if len(rows):
                out[rows] = res.results[e]["yT"].T[:len(rows)].astype(
                    np.float32)
    return out


# revision 21
# speedup vs baseline: 1.6202x; 1.2526x over previous
"""Expert-parallel MoE FFN kernel for Trainium2 (8 NeuronCores).

Strategy: each of the 8 experts lives on its own core. Rows are routed
host-side (argsort by note_type_pos), padded to a uniform capacity C,
and shipped feature-major (transposed) so the device kernel is a pure
dense 2-layer MLP with the feature dimension on SBUF partitions:

    hT = relu(W1.T @ xT + b1)     [F, C]
    yT = W2.T @ hT + b2           [H, C]

x and the weights are shipped in bf16 (error ~3e-3 vs the 2e-2 gate;
half the HBM traffic of f32); biases, PSUM accumulation and the output
stay f32. Each logical block moves as ONE 3D-strided DMA descriptor —
descriptor issue costs ~600ns serialized on the DGE path regardless of
size, so many small descriptors throttle the startup ramp. Weights are
streamed through SBUF once (F blocked into 8 blocks of 512); xT and the
y accumulator stay resident. No collectives needed.
"""

import sys

sys.path.insert(0, "/opt/trn_rl_repo")

import numpy as np

import concourse.bass as bass
import concourse.mybir as mybir
from concourse import bacc
from concourse.tile import TileContext

H = 1024
F = 4096
N_EXPERTS = 8
P = 128
KH = H // P   # 8
KF = F // P   # 32
FB = 1024     # F block size (weights streamed per block)
NFB = F // FB  # 4
FC = FB // P  # 8


def _row_tiles(C):
    """Split C columns into equal chunks <=512 (multiples of 16)."""
    n = -(-C // 512)
    rw = -(-C // n)
    rw = ((rw + 15) // 16) * 16
    tiles = []
    s = 0
    while s < C:
        w = min(rw, C - s)
        tiles.append((s, w))
        s += w
    return tiles


def build_expert_kernel(C, reps=1, n_wu=6):
    """One expert's 2-layer MLP: xT [H, C] -> yT [H, C].

    w1 arrives host-packed as [P, NFB*FC*KH*128] (partition-major blocked:
    per partition p, element (fb, fc, k, j) = W1[k*128+p, fb*512+fc*128+j])
    so every load slice is contiguous per partition — no sub-512B runs.
    """
    f32 = mybir.dt.float32
    bf16 = mybir.dt.bfloat16
    nc = bacc.Bacc(None, target_bir_lowering=False)
    xT = nc.dram_tensor("xT", [H, C], bf16, kind="ExternalInput")
    w1 = nc.dram_tensor("w1", [P, NFB * FC * KH * P], bf16,
                        kind="ExternalInput")
    b1v = nc.dram_tensor("b1v", [P, KF], f32, kind="ExternalInput")
    w2 = nc.dram_tensor("w2", [F, H], bf16, kind="ExternalInput")
    b2v = nc.dram_tensor("b2v", [P, KH], f32, kind="ExternalInput")
    yT = nc.dram_tensor("yT", [H, C], bf16, kind="ExternalOutput")

    # partition-major views: one DMA descriptor per logical block
    w2r = w2.rearrange("(f p) h -> p f h", p=P)   # [P, KF, H]
    xTr = xT.rearrange("(k p) c -> p k c", p=P)   # [P, KH, C]
    yTr = yT.rearrange("(m p) c -> p m c", p=P)   # [P, KH, C]

    FBW = FC * KH * P  # 4096 packed elements per fb block

    tiles = _row_tiles(C)
    rwmax = max(t[1] for t in tiles)

    with TileContext(nc) as tc:
        with (
            tc.tile_pool(name="consts", bufs=1) as consts,
            tc.tile_pool(name="xp", bufs=1) as xp,
            tc.tile_pool(name="yaccp", bufs=1) as yaccp,
            tc.tile_pool(name="youtp", bufs=1) as youtp,
            tc.tile_pool(name="w1p", bufs=2) as w1p,
            tc.tile_pool(name="w2p", bufs=2) as w2p,
            tc.tile_pool(name="hp", bufs=3) as hp,
            tc.tile_pool(name="psh", bufs=4, space="PSUM") as psh,
            tc.tile_pool(name="psy", bufs=4, space="PSUM") as psy,
        ):
            # Startup critical path, in consumption order: the first psum
            # group needs w1[fb0, fc0] + xT tile0 only; later chunks land
            # just ahead of the PE groups that consume them.
            r0_, rw_ = tiles[0]
            KHP = KH * P
            w1blk0 = w1p.tile([P, FBW], bf16, tag="w1blk")
            nc.sync.dma_start(w1blk0[:, 0:KHP], w1[:, 0:KHP])
            xT_sb = xp.tile([P, KH, C], bf16, tag="xT")
            # tile0 lands in two k-halves so the first psum groups can
            # start on k0-3 while k4-7 is still in flight
            nc.sync.dma_start(xT_sb[:, 0:KH // 2, r0_:r0_ + rw_],
                              xTr[:, 0:KH // 2, r0_:r0_ + rw_])
            nc.sync.dma_start(xT_sb[:, KH // 2:KH, r0_:r0_ + rw_],
                              xTr[:, KH // 2:KH, r0_:r0_ + rw_])
            nc.sync.dma_start(w1blk0[:, KHP:2 * KHP], w1[:, KHP:2 * KHP])
            b1_sb = consts.tile([P, KF], f32, tag="b1")
            nc.sync.dma_start(b1_sb[:], b1v[:, :])
            b2_sb = consts.tile([P, KH], f32, tag="b2")
            nc.sync.dma_start(b2_sb[:], b2v[:, :])
            for c0, c1 in ((2, 4), (4, 6), (6, FC)):
                nc.sync.dma_start(w1blk0[:, c0 * KHP:c1 * KHP],
                                  w1[:, c0 * KHP:c1 * KHP])
            w2blk0 = w2p.tile([P, FC, H], bf16, tag="w2blk")
            nc.sync.dma_start(w2blk0[:, :, 0:H // 2],
                              w2r[:, 0:FC, 0:H // 2])
            nc.sync.dma_start(w2blk0[:, :, H // 2:H],
                              w2r[:, 0:FC, H // 2:H])
            for (r0, rw) in tiles[1:]:
                nc.sync.dma_start(xT_sb[:, :, r0:r0 + rw],
                                  xTr[:, :, r0:r0 + rw])

            def load_w1(fb):
                w1blk = w1p.tile([P, FBW], bf16, tag="w1blk")
                nc.sync.dma_start(w1blk[:, :],
                                  w1[:, fb * FBW:(fb + 1) * FBW])
                return w1blk

            def load_w2(fb):
                w2blk = w2p.tile([P, FC, H], bf16, tag="w2blk")
                nc.sync.dma_start(w2blk[:, :, :],
                                  w2r[:, fb * FC:(fb + 1) * FC, :])
                return w2blk

            # PE warmup during the startup DMA window: absorbs the
            # p-state ramp so real matmuls start at full clock
            if n_wu:
                wu = consts.tile([P, 512], bf16, tag="wu")
                nc.vector.memset(wu[:], 0.0)
                wups = psh.tile([P, 512], f32, tag="ph")
                for i in range(n_wu):
                    nc.tensor.matmul(wups[:], wu[:, 0:P], wu[:],
                                     start=(i == 0), stop=(i == n_wu - 1))
                # touch Relu now so the act-table load (~1.3us) happens
                # inside the DMA window, not on the first real activation
                wuact = consts.tile([P, 16], f32, tag="wuact")
                nc.scalar.activation(wuact[:], wups[:, 0:16],
                                     mybir.ActivationFunctionType.Relu)

            yacc = yaccp.tile([P, KH, C], f32, tag="yacc")
            yout = youtp.tile([P, KH, C], bf16, tag="yout")

            def layer1(fb, w1blk, r0, rw, khalf_major=False):
                h_sb = hp.tile([P, FC, rwmax], bf16, tag="h")
                if khalf_major:
                    # startup path: sweep k0-3 over the first 4 fc groups
                    # (all psh banks) so the PE isn't gated on the second
                    # xT half; the remaining fc run the normal loop
                    nkh = min(FC, 4)
                    phs = []
                    for fci in range(nkh):
                        ph_i = psh.tile([P, rw], f32, tag="ph",
                                        name=f"ph_s{fci}")
                        phs.append(ph_i)
                    for kh in range(2):
                        for fc in range(nkh):
                            for k in range(kh * KH // 2,
                                           (kh + 1) * KH // 2):
                                o = fc * KHP + k * P
                                nc.tensor.matmul(
                                    phs[fc][:],
                                    w1blk[:, o:o + P],
                                    xT_sb[:, k, r0:r0 + rw],
                                    start=(k == 0), stop=(k == KH - 1))
                    for fc in range(nkh):
                        nc.scalar.activation(
                            h_sb[:, fc, :rw], phs[fc][:],
                            mybir.ActivationFunctionType.Relu,
                            bias=b1_sb[:, fb * FC + fc:fb * FC + fc + 1])
                else:
                    nkh = 0
                for fc in range(nkh, FC):
                    ph = psh.tile([P, rw], f32, tag="ph")
                    for k in range(KH):
                        o = fc * KHP + k * P
                        nc.tensor.matmul(
                            ph[:],
                            w1blk[:, o:o + P],
                            xT_sb[:, k, r0:r0 + rw],
                            start=(k == 0), stop=(k == KH - 1))
                    nc.scalar.activation(
                        h_sb[:, fc, :rw], ph[:],
                        mybir.ActivationFunctionType.Relu,
                        bias=b1_sb[:, fb * FC + fc:fb * FC + fc + 1])
                return h_sb

            def layer2(fb, w2blk, h_sb, m, r0, rw, last, final_tile):
                py = psy.tile([P, rw], f32, tag="py")
                for fc in range(FC):
                    nc.tensor.matmul(
                        py[:],
                        w2blk[:, fc, m * P:(m + 1) * P],
                        h_sb[:, fc, :rw],
                        start=(fc == 0), stop=(fc == FC - 1))
                if fb == 0:
                    # fold the layer-2 bias into the first partial
                    nc.scalar.activation(
                        yacc[:, m, r0:r0 + rw], py[:],
                        mybir.ActivationFunctionType.Identity,
                        bias=b2_sb[:, m:m + 1])
                elif fb < NFB - 1:
                    nc.vector.tensor_add(
                        out=yacc[:, m, r0:r0 + rw],
                        in0=yacc[:, m, r0:r0 + rw], in1=py[:])
                else:
                    # final accumulation narrows to the bf16 output stage
                    nc.vector.tensor_add(
                        out=yout[:, m, r0:r0 + rw],
                        in0=yacc[:, m, r0:r0 + rw], in1=py[:])
                    if not last:
                        return
                    # stream writebacks behind the adds; the final tile
                    # tapers (half, then 3, then 1) so the last descriptor
                    # after the last add is as small as possible
                    if final_tile and m in (3, 6, 7):
                        m0 = {3: 0, 6: 4, 7: 7}[m]
                        nc.sync.dma_start(
                            yTr[:, m0:m + 1, r0:r0 + rw],
                            yout[:, m0:m + 1, r0:r0 + rw])
                    elif not final_tile and m in (KH // 2 - 1, KH - 1):
                        m0 = 0 if m == KH // 2 - 1 else KH // 2
                        nc.sync.dma_start(
                            yTr[:, m0:m + 1, r0:r0 + rw],
                            yout[:, m0:m + 1, r0:r0 + rw])

            def body(first_blks=None, last=True):
                for fb in range(NFB):
                    if fb == 0 and first_blks is not None:
                        w1blk, w2blk = first_blks
                    else:
                        w1blk = load_w1(fb)
                        w2blk = load_w2(fb)
                    for ti, (r0, rw) in enumerate(tiles):
                        h_sb = layer1(fb, w1blk, r0, rw,
                                      khalf_major=(fb == 0 and ti == 0
                                                   and first_blks
                                                   is not None))
                        for m in range(KH):
                            layer2(fb, w2blk, h_sb, m, r0, rw, last,
                                   ti == len(tiles) - 1)

            first_blks = (w1blk0, w2blk0)
            for i in range(reps - 1):
                body(first_blks if i == 0 else None, last=False)
            body(first_blks if reps == 1 else None, last=True)
    nc.finalize()
    return nc


# SBUF residency (xT bf16 + yacc f32 at 48*C B/partition) caps capacity.
MAX_C = 1536


def _prepare(x, note_type_pos, W1, b1, W2, b2, cap):
    """Host-side routing: sort rows by expert, pad to capacity C (<= cap)."""
    import ml_dtypes
    bf16 = ml_dtypes.bfloat16
    ntp = np.asarray(note_type_pos).astype(np.int64)
    x = np.ascontiguousarray(np.asarray(x, dtype=np.float32))
    counts = np.bincount(ntp, minlength=N_EXPERTS)
    C = min(int(counts.max()), cap)
    C = max(16, ((C + 15) // 16) * 16)  # 16-aligned, no extra row-tile padding

    order = np.argsort(ntp, kind="stable")
    weights = []
    for e in range(N_EXPERTS):
        # pack W1 partition-major blocked: [P, (fb, fc, k, j)] so device
        # loads are contiguous per partition at any chunking granularity
        w1p_ = (np.asarray(W1[e]).astype(bf16)
                .reshape(KH, P, NFB, FC, P)
                .transpose(1, 2, 3, 0, 4)
                .reshape(P, NFB * FC * KH * P))
        weights.append({
            "w1": np.ascontiguousarray(w1p_),
            "b1v": np.ascontiguousarray(
                np.asarray(b1[e], dtype=np.float32).reshape(KF, P).T),
            "w2": np.ascontiguousarray(np.asarray(W2[e]).astype(bf16)),
            "b2v": np.ascontiguousarray(
                np.asarray(b2[e], dtype=np.float32).reshape(KH, P).T),
        })
    # chunk each expert's rows into groups of <= C; one SPMD launch per group
    launches = []
    off = 0
    expert_rows = []
    for e in range(N_EXPERTS):
        expert_rows.append(order[off:off + counts[e]])
        off += counts[e]
    n_launch = max(1, -(-int(counts.max()) // C))
    for g in range(n_launch):
        in_maps, row_idx = [], []
        for e in range(N_EXPERTS):
            rows = expert_rows[e][g * C:(g + 1) * C]
            row_idx.append(rows)
            xe = np.zeros((C, H), dtype=np.float32)
            if len(rows):
                xe[:len(rows)] = x[rows]
            in_maps.append({"xT": np.ascontiguousarray(xe.T.astype(bf16)),
                            **weights[e]})
        launches.append((in_maps, row_idx))
    return launches, C


def kernel(x, note_type_pos, W1, b1, W2, b2):
    launches, C = _prepare(x, note_type_pos, W1, b1, W2, b2, cap=MAX_C)
    nc = build_expert_kernel(C)
    from concourse.bass_utils import run_bass_kernel_spmd
    T = np.asarray(x).shape[0]
    out = np.zeros((T, H), dtype=np.float32)
    for in_maps, row_idx in launches:
        res = run_bass_kernel_spmd(nc, in_maps, core_ids=list(range(N_EXPERTS)))
        for e in range(N_EXPERTS):
            rows = row_idx[e]
            if len(rows):
                out[rows] = res.results[e]["yT"].T[:len(rows)].astype(
                    np.float32)
    return out
